# revision 6
# baseline (speedup 1.0000x reference)
"""CompressedLinear kernel for 8 TRN2 NeuronCores.

out[B,S,DOUT] = x[B,S,DIN] @ (w_int8 * scale).T + bias

Strategy (tensor-parallel, per sharding hint):
  - Shard weight rows (DOUT=11008) across 8 cores -> 1376 rows/core.
  - Replicate x to all cores.
  - Host-side prep: fold `scale` into x, cast x and w to fp16 (int8 codes
    <= 127 are exact in fp16), and pre-transpose both operands into
    K-major layouts so every DMA is contiguous per partition line.
  - On-chip: out_tile[128 tok, n] accumulates over K in PSUM via
    matmul(lhsT=xT_tile[128k, 128tok], rhs=wT_tile[128k, n]); epilogue is
    one DVE add (psum + bias_broadcast -> SBUF fp32), then DMA out.
  - Gather: concat per-core outputs along the feature axis on host.

Head/tail scheduling (the PE is >98% busy in steady state, so the only
wins left are at the edges):
  - First-dependency DMAs (k0:2 slices of x-tiles 0/1, n0 slice of w
    chunk 0) are issued from the Scalar/GpSimd/Vector queues, which come
    out of engine boot ~1us before the Sync queue does; the bulk streams
    follow on Sync.  The leading two token tiles consume k0/k1 in
    n-major order so compute can start as soon as those slices land.
  - PE warmup (HAM clock-gate ramp) uses a small memset so it starts as
    early as possible.
  - The last token tile runs n-major with per-n epilogue+store so the
    final store is only the 352-wide slice.
"""

import sys
import types

import numpy as np

import concourse.mybir as mybir
import concourse.tile as tile
from concourse import bacc
from concourse.bass_utils import run_bass_kernel_spmd


def _ensure_ntff_hook():
    """Some images lack antenv.axon_hooks; run_bass_kernel_spmd imports it
    on the traced path (e.g. if BASS_TRACE is set in the environment).
    Register a working shim backed by the axon .so when possible, else a
    no-op getter, so tracing degrades gracefully instead of crashing."""
    try:
        import antenv.axon_hooks  # noqa: F401
        return
    except ImportError:
        pass
    hook = None
    try:
        from trn_agent_boot.trn_boot import _ntff_profile_via_ctypes

        hook = _ntff_profile_via_ctypes("/opt/axon/libaxon_pjrt.so")
    except Exception:
        hook = None
    mod = types.ModuleType("antenv.axon_hooks")
    mod.get_axon_ntff_profile_hook = lambda: hook
    mod.set_axon_ntff_profile_hook = lambda h: None
    sys.modules["antenv.axon_hooks"] = mod


_ensure_ntff_hook()

# Problem shapes (hardcoded per contract)
B, S, DIN, DOUT = 2, 2048, 4096, 11008
NCORES = 8
TOK = B * S                      # 4096 tokens
DSH = DOUT // NCORES             # 1376 output features per core
P = 128
KC = DIN // P                    # 32 contraction chunks of 128
MT = TOK // P                    # 32 token tiles of 128
N_TILE = 512
N_SIZES = (512, 512, 352)        # n-tiles covering DSH=1376
HEAD_KC = 2                      # leading k-slices loaded via early queues
WARM_N = 256                     # warmup matmul width
WARM_COUNT = 18                  # number of warmup matmuls

_cached = {}


def build_module(mt=MT, kc=KC, dsh=DSH, n_sizes=N_SIZES, num_devices=NCORES):
    """Build + compile the Bass module (same NEFF for all cores)."""
    nc = bacc.Bacc(
        "TRN2",
        target_bir_lowering=False,
        debug=False,
        num_devices=num_devices,
    )
    fp16 = mybir.dt.float16
    fp32 = mybir.dt.float32

    # DRAM I/O (per-core shapes; layouts pre-arranged on host)
    x_d = nc.dram_tensor("x", (mt, P, kc, P), fp16, kind="ExternalInput")
    w_d = nc.dram_tensor("w", (P, kc, dsh), fp16, kind="ExternalInput")
    b_d = nc.dram_tensor("b", (P, dsh), fp32, kind="ExternalInput")
    o_d = nc.dram_tensor("out", (mt, P, dsh), fp32, kind="ExternalOutput")

    n_off = []
    off = 0
    for ns in n_sizes:
        n_off.append(off)
        off += ns
    assert off == dsh

    # Weight DMA chunk boundaries (in kc units): uniform 2-kc chunks.
    # (A smaller first chunk and other head perturbations measured slower.)
    step = 2 if kc % 2 == 0 else 1
    w_bounds = list(range(0, kc + 1, step))
    # kc index -> (chunk index, offset within chunk)
    k2chunk = []
    for ci in range(len(w_bounds) - 1):
        for kk in range(w_bounds[ci + 1] - w_bounds[ci]):
            k2chunk.append((ci, kk))

    # How many leading token-tiles to k-interleave so PE work overlaps the
    # weight load (each tile is ~18.3us of PE work vs ~30us of w DMA).
    n_group = 2 if mt >= 2 else mt
    hkc = HEAD_KC

    with tile.TileContext(nc) as tc:
        with (
            tc.tile_pool(name="wpool", bufs=1) as wpool,
            tc.tile_pool(name="xpool", bufs=4) as xpool,
            tc.tile_pool(name="opool", bufs=3) as opool,
            tc.tile_pool(name="psum", bufs=2, space="PSUM") as psum_pool,
        ):
            # ---- early head DMAs on the non-Sync queues --------------------
            # k0:hkc slices of x tiles 0/1 and the n0 slice of w's first
            # chunk are the gate for the first real matmul; Scalar/GpSimd
            # come out of boot before Sync's first DMA issues.
            w0n_tiles = []
            w0n0 = wpool.tile([P, hkc, n_sizes[0]], fp16, tag="w0n0")
            nc.scalar.dma_start(out=w0n0[:], in_=w_d.ap()[:, 0:hkc, 0 : n_sizes[0]])
            w0n_tiles.append(w0n0)

            xheads = []
            for g in range(n_group):
                xh = wpool.tile([P, hkc, P], fp16, tag=f"x{g}h")
                nc.gpsimd.dma_start(out=xh[:], in_=x_d.ap()[g][:, 0:hkc, :])
                xheads.append(xh)

            # PE warmup: dummy matmuls on a small zeroed scratch tile so the
            # HAM clock-gate ramps while the head DMAs are in flight.
            warm_src = wpool.tile([P, WARM_N], fp16, tag="warm_src")
            nc.vector.memset(warm_src[:], 0)
            warm_ps = psum_pool.tile([P, WARM_N], fp32, tag="warm", name="warm")
            for _ in range(WARM_COUNT):
                nc.tensor.matmul(
                    warm_ps[:], warm_src[:, :P], warm_src[:], start=True, stop=True
                )

            # rest of w chunk 0 (n1+n2), second on the Scalar queue
            w0n12 = wpool.tile([P, hkc, dsh - n_sizes[0]], fp16, tag="w0n12")
            nc.scalar.dma_start(out=w0n12[:], in_=w_d.ap()[:, 0:hkc, n_sizes[0] : dsh])
            w0n_tiles.append(w0n12)

            # ---- bulk streams on Sync --------------------------------------
            def alloc_xm(m, head=False):
                if head:
                    xm = xpool.tile([P, kc - hkc, P], fp16, tag="xm", name=f"xm{m}")
                    nc.sync.dma_start(out=xm[:], in_=x_d.ap()[m][:, hkc:kc, :])
                else:
                    xm = xpool.tile([P, kc, P], fp16, tag="xm", name=f"xm{m}")
                    nc.sync.dma_start(out=xm[:], in_=x_d.ap()[m])
                return xm

            def alloc_psums(m):
                psums = []
                for n in range(len(n_sizes)):
                    ps_full = psum_pool.tile(
                        [P, N_TILE], fp32, tag=f"ps{n}", name=f"ps{n}_{m}"
                    )
                    psums.append(ps_full[:, : n_sizes[n]])
                return psums

            def w_slice(wt, kk, n):
                """Weight AP for (chunk tile, offset kk, n-tile). Chunk 0
                lives in the two head tiles, split at n_sizes[0]."""
                if wt is None:
                    if n == 0:
                        return w0n_tiles[0][:, kk, :]
                    return w0n_tiles[1][
                        :, kk, n_off[n] - n_sizes[0] : n_off[n] - n_sizes[0] + n_sizes[n]
                    ]
                return wt[:, kk, n_off[n] : n_off[n] + n_sizes[n]]

            def mm_lhsT(psums, lhsT, k, wt, kk):
                for n in range(len(n_sizes)):
                    nc.tensor.matmul(
                        psums[n],
                        lhsT,
                        w_slice(wt, kk, n),
                        start=(k == 0),
                        stop=(k == kc - 1),
                    )

            def epilogue(m, psums, split_store=False):
                om = opool.tile([P, dsh], fp32, tag="om", name=f"om{m}")
                for n in range(len(n_sizes)):
                    sl = slice(n_off[n], n_off[n] + n_sizes[n])
                    nc.vector.tensor_add(
                        out=om[:, sl], in0=psums[n], in1=bias_sb[:, sl]
                    )
                    if split_store:
                        nc.sync.dma_start(out=o_d.ap()[m][:, sl], in_=om[:, sl])
                if not split_store:
                    nc.sync.dma_start(out=o_d.ap()[m], in_=om[:])

            # DMA issue order on Sync is FIFO: x rest tiles for the leading
            # group, then remaining weight chunks in consumption order; bias
            # is only needed by the first epilogue (~45us in), so it's last.
            w_tiles = [None]  # chunk 0 handled by the head tiles

            def load_w_chunk(c):
                lo, hi = w_bounds[c], w_bounds[c + 1]
                wt = wpool.tile([P, hi - lo, dsh], fp16, tag=f"w{c}", name=f"w{c}")
                nc.sync.dma_start(out=wt[:], in_=w_d.ap()[:, lo:hi, :])
                w_tiles.append(wt)

            group_xms = [alloc_xm(g, head=True) for g in range(n_group)]
            for c in range(1, len(w_bounds) - 1):
                load_w_chunk(c)

            bias_sb = wpool.tile([P, dsh], fp32, tag="bias")
            nc.sync.dma_start(out=bias_sb[:], in_=b_d.ap())

            # Leading group, k < hkc: n-major so compute starts on the
            # (w0n0, xheads) slices before the rest of chunk 0 lands.
            group_psums = [alloc_psums(m) for m in range(n_group)]
            for n in range(len(n_sizes)):
                if n == 0:
                    wt, noff = w0n_tiles[0], 0
                else:
                    wt, noff = w0n_tiles[1], n_off[n] - n_sizes[0]
                for k in range(hkc):
                    for g in range(n_group):
                        nc.tensor.matmul(
                            group_psums[g][n],
                            xheads[g][:, k, :],
                            wt[:, k, noff : noff + n_sizes[n]],
                            start=(k == 0),
                            stop=False,
                        )
            # Leading group, k >= hkc: interleave over k so matmuls consume
            # weight chunks in arrival order across n_group token tiles.
            for k in range(hkc, kc):
                ci, kk = k2chunk[k]
                wt = w_tiles[ci]
                for g in range(n_group):
                    mm_lhsT(group_psums[g], group_xms[g][:, k - hkc, :], k, wt, kk)
            for g in range(n_group):
                epilogue(g, group_psums[g])

            # Steady state
            for m in range(n_group, mt - 1):
                xm = alloc_xm(m)
                psums = alloc_psums(m)
                for k in range(kc):
                    ci, kk = k2chunk[k]
                    mm_lhsT(psums, xm[:, k, :], k, w_tiles[ci], kk)
                epilogue(m, psums)

            # Last tile: n-major with per-n epilogue+store so the tail after
            # the final matmul is only the 352-wide add + store.
            m = mt - 1
            xm = alloc_xm(m)
            psums = alloc_psums(m)
            om = opool.tile([P, dsh], fp32, tag="om", name=f"om{m}")
            for n in range(len(n_sizes)):
                sl = slice(n_off[n], n_off[n] + n_sizes[n])
                for k in range(kc):
                    ci, kk = k2chunk[k]
                    nc.tensor.matmul(
                        psums[n],
                        xm[:, k, :],
                        w_slice(w_tiles[ci], kk, n),
                        start=(k == 0),
                        stop=(k == kc - 1),
                    )
                nc.vector.tensor_add(out=om[:, sl], in0=psums[n], in1=bias_sb[:, sl])
                nc.scalar.dma_start(out=o_d.ap()[m][:, sl], in_=om[:, sl])

    nc.compile()
    return nc


def _get_module():
    if "nc" not in _cached:
        # num_devices=1: no collectives anywhere in the kernel, and the
        # per-NEFF sync machinery is cheapest in single-device form; the
        # SPMD launcher still runs the same NEFF on all 8 cores.
        _cached["nc"] = build_module(num_devices=1)
    return _cached["nc"]


def _prep_inputs(x, w_int8, scale, bias):
    """Host-side shard + layout prep. Returns in_maps for the 8 cores."""
    # x: fold scale, cast fp16, reorder to [m, kp, kc, t]
    xs = x.reshape(TOK, DIN).astype(np.float32) * np.float32(scale)
    xp = xs.reshape(MT, P, KC, P)        # [m, t, kc, kp]
    xp = np.ascontiguousarray(xp.transpose(0, 3, 2, 1), dtype=np.float16)

    in_maps = []
    for c in range(NCORES):
        wsh = w_int8[c * DSH : (c + 1) * DSH]          # [dsh, DIN] int32
        wp = wsh.reshape(DSH, KC, P).transpose(2, 1, 0)  # [kp, kc, dsh]
        wp = np.ascontiguousarray(wp).astype(np.float16)  # ints <=127: exact
        bsh = np.ascontiguousarray(
            np.broadcast_to(bias[c * DSH : (c + 1) * DSH].astype(np.float32), (P, DSH))
        )
        in_maps.append({"x": xp, "w": wp, "b": bsh})
    return in_maps


def kernel(x, w_int8, scale, bias):
    nc = _get_module()
    in_maps = _prep_inputs(
        np.asarray(x), np.asarray(w_int8), np.asarray(scale), np.asarray(bias)
    )
    res = run_bass_kernel_spmd(nc, in_maps, core_ids=list(range(NCORES)))
    outs = [res.results[c]["out"].reshape(TOK, DSH) for c in range(NCORES)]
    full = np.concatenate(outs, axis=1)  # [TOK, DOUT]
    return np.ascontiguousarray(full.reshape(B, S, DOUT), dtype=np.float32)


# revision 8
# speedup vs baseline: 1.0012x; 1.0012x over previous
"""CompressedLinear kernel for 8 TRN2 NeuronCores.

out[B,S,DOUT] = x[B,S,DIN] @ (w_int8 * scale).T + bias

Strategy (tensor-parallel, per sharding hint):
  - Shard weight rows (DOUT=11008) across 8 cores -> 1376 rows/core.
  - Replicate x to all cores.
  - Host-side prep: fold `scale` into x, cast x and w to fp16 (int8 codes
    <= 127 are exact in fp16), and pre-transpose both operands into
    K-major layouts so every DMA is contiguous per partition line.
  - On-chip: out_tile[128 tok, n] accumulates over K in PSUM via
    matmul(lhsT=xT_tile[128k, 128tok], rhs=wT_tile[128k, n]); epilogue is
    one DVE add (psum + bias_broadcast -> SBUF fp32), then DMA out.
  - Gather: concat per-core outputs along the feature axis on host.

Head/tail scheduling (the PE is >98% busy in steady state, so the only
wins left are at the edges):
  - First-dependency DMAs (k0:2 slices of x-tiles 0/1, n0 slice of w
    chunk 0) are issued from the Scalar/GpSimd/Vector queues, which come
    out of engine boot ~1us before the Sync queue does; the bulk streams
    follow on Sync.  The leading two token tiles consume k0/k1 in
    n-major order so compute can start as soon as those slices land.
  - PE warmup (HAM clock-gate ramp) uses a small memset so it starts as
    early as possible.
  - The last token tile runs n-major with per-n epilogue+store so the
    final store is only the 352-wide slice.
"""

import sys
import types

import numpy as np

import concourse.mybir as mybir
import concourse.tile as tile
from concourse import bacc
from concourse.bass_utils import run_bass_kernel_spmd


def _ensure_ntff_hook():
    """Some images lack antenv.axon_hooks; run_bass_kernel_spmd imports it
    on the traced path (e.g. if BASS_TRACE is set in the environment).
    Register a working shim backed by the axon .so when possible, else a
    no-op getter, so tracing degrades gracefully instead of crashing."""
    try:
        import antenv.axon_hooks  # noqa: F401
        return
    except ImportError:
        pass
    hook = None
    try:
        from trn_agent_boot.trn_boot import _ntff_profile_via_ctypes

        hook = _ntff_profile_via_ctypes("/opt/axon/libaxon_pjrt.so")
    except Exception:
        hook = None
    mod = types.ModuleType("antenv.axon_hooks")
    mod.get_axon_ntff_profile_hook = lambda: hook
    mod.set_axon_ntff_profile_hook = lambda h: None
    sys.modules["antenv.axon_hooks"] = mod


_ensure_ntff_hook()

# Problem shapes (hardcoded per contract)
B, S, DIN, DOUT = 2, 2048, 4096, 11008
NCORES = 8
TOK = B * S                      # 4096 tokens
DSH = DOUT // NCORES             # 1376 output features per core
P = 128
KC = DIN // P                    # 32 contraction chunks of 128
MT = TOK // P                    # 32 token tiles of 128
N_TILE = 512
N_SIZES = (512, 512, 352)        # n-tiles covering DSH=1376
HEAD_KC = 2                      # leading k-slices loaded via early queues
WARM_N = 256                     # warmup matmul width
WARM_COUNT = 10                  # number of warmup matmuls

_cached = {}


def build_module(mt=MT, kc=KC, dsh=DSH, n_sizes=N_SIZES, num_devices=NCORES):
    """Build + compile the Bass module (same NEFF for all cores)."""
    nc = bacc.Bacc(
        "TRN2",
        target_bir_lowering=False,
        debug=False,
        num_devices=num_devices,
    )
    fp16 = mybir.dt.float16
    fp32 = mybir.dt.float32

    # DRAM I/O (per-core shapes; layouts pre-arranged on host)
    x_d = nc.dram_tensor("x", (mt, P, kc, P), fp16, kind="ExternalInput")
    w_d = nc.dram_tensor("w", (P, kc, dsh), fp16, kind="ExternalInput")
    b_d = nc.dram_tensor("b", (P, dsh), fp32, kind="ExternalInput")
    o_d = nc.dram_tensor("out", (mt, P, dsh), fp32, kind="ExternalOutput")

    n_off = []
    off = 0
    for ns in n_sizes:
        n_off.append(off)
        off += ns
    assert off == dsh

    # Weight DMA chunk boundaries (in kc units): uniform 2-kc chunks.
    # (A smaller first chunk and other head perturbations measured slower.)
    step = 2 if kc % 2 == 0 else 1
    w_bounds = list(range(0, kc + 1, step))
    # kc index -> (chunk index, offset within chunk)
    k2chunk = []
    for ci in range(len(w_bounds) - 1):
        for kk in range(w_bounds[ci + 1] - w_bounds[ci]):
            k2chunk.append((ci, kk))

    # How many leading token-tiles to k-interleave so PE work overlaps the
    # weight load (each tile is ~18.3us of PE work vs ~30us of w DMA).
    n_group = 2 if mt >= 2 else mt
    hkc = HEAD_KC

    with tile.TileContext(nc) as tc:
        with (
            tc.tile_pool(name="wpool", bufs=1) as wpool,
            tc.tile_pool(name="xpool", bufs=4) as xpool,
            tc.tile_pool(name="opool", bufs=3) as opool,
            tc.tile_pool(name="psum", bufs=2, space="PSUM") as psum_pool,
        ):
            # ---- head ------------------------------------------------------
            # PE warmup: dummy matmuls on a small zeroed scratch tile so the
            # HAM clock-gate ramps while the head DMAs are in flight.
            warm_src = wpool.tile([P, WARM_N], fp16, tag="warm_src")
            nc.gpsimd.memset(warm_src[:], 0)
            warm_ps = psum_pool.tile([P, WARM_N], fp32, tag="warm", name="warm")
            for _ in range(WARM_COUNT):
                nc.tensor.matmul(
                    warm_ps[:], warm_src[:, :P], warm_src[:], start=True, stop=True
                )

            # First-dependency DMAs, all on Sync (its queue group fans out
            # across 16 HW engines at ~280GB/s; the Scalar/GpSimd queue
            # groups measured ~30GB/s and are useless for bulk).  Gate for
            # the first real matmul = x0h + x1h + w chunk 0 = 832KB.
            xheads = []
            for g in range(n_group):
                xh = wpool.tile([P, hkc, P], fp16, tag=f"x{g}h")
                nc.sync.dma_start(out=xh[:], in_=x_d.ap()[g][:, 0:hkc, :])
                xheads.append(xh)
            w0full = wpool.tile([P, hkc, dsh], fp16, tag="w0")
            nc.sync.dma_start(out=w0full[:], in_=w_d.ap()[:, 0:hkc, :])

            # ---- bulk streams on Sync --------------------------------------
            def alloc_xm(m, head=False):
                if head:
                    xm = xpool.tile([P, kc - hkc, P], fp16, tag="xm", name=f"xm{m}")
                    nc.sync.dma_start(out=xm[:], in_=x_d.ap()[m][:, hkc:kc, :])
                else:
                    xm = xpool.tile([P, kc, P], fp16, tag="xm", name=f"xm{m}")
                    nc.sync.dma_start(out=xm[:], in_=x_d.ap()[m])
                return xm

            def alloc_psums(m):
                psums = []
                for n in range(len(n_sizes)):
                    ps_full = psum_pool.tile(
                        [P, N_TILE], fp32, tag=f"ps{n}", name=f"ps{n}_{m}"
                    )
                    psums.append(ps_full[:, : n_sizes[n]])
                return psums

            def w_slice(wt, kk, n):
                return wt[:, kk, n_off[n] : n_off[n] + n_sizes[n]]

            def mm_lhsT(psums, lhsT, k, wt, kk):
                for n in range(len(n_sizes)):
                    nc.tensor.matmul(
                        psums[n],
                        lhsT,
                        w_slice(wt, kk, n),
                        start=(k == 0),
                        stop=(k == kc - 1),
                    )

            def epilogue(m, psums, split_store=False):
                om = opool.tile([P, dsh], fp32, tag="om", name=f"om{m}")
                for n in range(len(n_sizes)):
                    sl = slice(n_off[n], n_off[n] + n_sizes[n])
                    nc.vector.tensor_add(
                        out=om[:, sl], in0=psums[n], in1=bias_sb[:, sl]
                    )
                    if split_store:
                        nc.sync.dma_start(out=o_d.ap()[m][:, sl], in_=om[:, sl])
                if not split_store:
                    nc.sync.dma_start(out=o_d.ap()[m], in_=om[:])

            # DMA issue order on Sync is FIFO: x rest tiles for the leading
            # group, then remaining weight chunks in consumption order; bias
            # is only needed by the first epilogue (~45us in), so it's last.
            w_tiles = [w0full]

            def load_w_chunk(c):
                lo, hi = w_bounds[c], w_bounds[c + 1]
                wt = wpool.tile([P, hi - lo, dsh], fp16, tag=f"w{c}", name=f"w{c}")
                nc.sync.dma_start(out=wt[:], in_=w_d.ap()[:, lo:hi, :])
                w_tiles.append(wt)

            group_xms = [alloc_xm(g, head=True) for g in range(n_group)]
            for c in range(1, len(w_bounds) - 1):
                load_w_chunk(c)

            bias_sb = wpool.tile([P, dsh], fp32, tag="bias")
            nc.sync.dma_start(out=bias_sb[:], in_=b_d.ap())

            # Leading group, k < hkc: gated only on the head tiles + chunk 0.
            group_psums = [alloc_psums(m) for m in range(n_group)]
            for k in range(hkc):
                for g in range(n_group):
                    for n in range(len(n_sizes)):
                        nc.tensor.matmul(
                            group_psums[g][n],
                            xheads[g][:, k, :],
                            w_slice(w0full, k, n),
                            start=(k == 0),
                            stop=False,
                        )
            # Leading group, k >= hkc: interleave over k so matmuls consume
            # weight chunks in arrival order across n_group token tiles.
            for k in range(hkc, kc):
                ci, kk = k2chunk[k]
                wt = w_tiles[ci]
                for g in range(n_group):
                    mm_lhsT(group_psums[g], group_xms[g][:, k - hkc, :], k, wt, kk)
            for g in range(n_group):
                epilogue(g, group_psums[g])

            # Steady state
            for m in range(n_group, mt - 1):
                xm = alloc_xm(m)
                psums = alloc_psums(m)
                for k in range(kc):
                    ci, kk = k2chunk[k]
                    mm_lhsT(psums, xm[:, k, :], k, w_tiles[ci], kk)
                epilogue(m, psums)

            # Last tile: n-major with per-n epilogue+store so the tail after
            # the final matmul is only the 352-wide add + store.
            m = mt - 1
            xm = alloc_xm(m)
            psums = alloc_psums(m)
            om = opool.tile([P, dsh], fp32, tag="om", name=f"om{m}")
            for n in range(len(n_sizes)):
                sl = slice(n_off[n], n_off[n] + n_sizes[n])
                for k in range(kc):
                    ci, kk = k2chunk[k]
                    nc.tensor.matmul(
                        psums[n],
                        xm[:, k, :],
                        w_slice(w_tiles[ci], kk, n),
                        start=(k == 0),
                        stop=(k == kc - 1),
                    )
                nc.vector.tensor_add(out=om[:, sl], in0=psums[n], in1=bias_sb[:, sl])
                nc.scalar.dma_start(out=o_d.ap()[m][:, sl], in_=om[:, sl])

    nc.compile()
    return nc


def _get_module():
    if "nc" not in _cached:
        # num_devices=1: no collectives anywhere in the kernel, and the
        # per-NEFF sync machinery is cheapest in single-device form; the
        # SPMD launcher still runs the same NEFF on all 8 cores.
        _cached["nc"] = build_module(num_devices=1)
    return _cached["nc"]


def _prep_inputs(x, w_int8, scale, bias):
    """Host-side shard + layout prep. Returns in_maps for the 8 cores."""
    # x: fold scale, cast fp16, reorder to [m, kp, kc, t]
    xs = x.reshape(TOK, DIN).astype(np.float32) * np.float32(scale)
    xp = xs.reshape(MT, P, KC, P)        # [m, t, kc, kp]
    xp = np.ascontiguousarray(xp.transpose(0, 3, 2, 1), dtype=np.float16)

    in_maps = []
    for c in range(NCORES):
        wsh = w_int8[c * DSH : (c + 1) * DSH]          # [dsh, DIN] int32
        wp = wsh.reshape(DSH, KC, P).transpose(2, 1, 0)  # [kp, kc, dsh]
        wp = np.ascontiguousarray(wp).astype(np.float16)  # ints <=127: exact
        bsh = np.ascontiguousarray(
            np.broadcast_to(bias[c * DSH : (c + 1) * DSH].astype(np.float32), (P, DSH))
        )
        in_maps.append({"x": xp, "w": wp, "b": bsh})
    return in_maps


def kernel(x, w_int8, scale, bias):
    nc = _get_module()
    in_maps = _prep_inputs(
        np.asarray(x), np.asarray(w_int8), np.asarray(scale), np.asarray(bias)
    )
    res = run_bass_kernel_spmd(nc, in_maps, core_ids=list(range(NCORES)))
    outs = [res.results[c]["out"].reshape(TOK, DSH) for c in range(NCORES)]
    full = np.concatenate(outs, axis=1)  # [TOK, DOUT]
    return np.ascontiguousarray(full.reshape(B, S, DOUT), dtype=np.float32)


# revision 9
# speedup vs baseline: 1.0053x; 1.0041x over previous
"""CompressedLinear kernel for 8 TRN2 NeuronCores.

out[B,S,DOUT] = x[B,S,DIN] @ (w_int8 * scale).T + bias

Strategy (tensor-parallel, per sharding hint):
  - Shard weight rows (DOUT=11008) across 8 cores -> 1376 rows/core.
  - Replicate x to all cores.
  - Host-side prep: fold `scale` into x, cast x and w to fp16 (int8 codes
    <= 127 are exact in fp16), and pre-transpose both operands into
    K-major layouts so every DMA is contiguous per partition line.
  - On-chip: out_tile[128 tok, n] accumulates over K in PSUM via
    matmul(lhsT=xT_tile[128k, 128tok], rhs=wT_tile[128k, n]); epilogue is
    one DVE add (psum + bias_broadcast -> SBUF fp32), then DMA out.
  - Gather: concat per-core outputs along the feature axis on host.

Head/tail scheduling (the PE is >98% busy in steady state, so the only
wins left are at the edges):
  - First-dependency DMAs (k0:2 slices of x-tiles 0/1, n0 slice of w
    chunk 0) are issued from the Scalar/GpSimd/Vector queues, which come
    out of engine boot ~1us before the Sync queue does; the bulk streams
    follow on Sync.  The leading two token tiles consume k0/k1 in
    n-major order so compute can start as soon as those slices land.
  - PE warmup (HAM clock-gate ramp) uses a small memset so it starts as
    early as possible.
  - The last token tile runs n-major with per-n epilogue+store so the
    final store is only the 352-wide slice.
"""

import sys
import types

import numpy as np

import concourse.mybir as mybir
import concourse.tile as tile
from concourse import bacc
from concourse.bass_utils import run_bass_kernel_spmd


def _ensure_ntff_hook():
    """Some images lack antenv.axon_hooks; run_bass_kernel_spmd imports it
    on the traced path (e.g. if BASS_TRACE is set in the environment).
    Register a working shim backed by the axon .so when possible, else a
    no-op getter, so tracing degrades gracefully instead of crashing."""
    try:
        import antenv.axon_hooks  # noqa: F401
        return
    except ImportError:
        pass
    hook = None
    try:
        from trn_agent_boot.trn_boot import _ntff_profile_via_ctypes

        hook = _ntff_profile_via_ctypes("/opt/axon/libaxon_pjrt.so")
    except Exception:
        hook = None
    mod = types.ModuleType("antenv.axon_hooks")
    mod.get_axon_ntff_profile_hook = lambda: hook
    mod.set_axon_ntff_profile_hook = lambda h: None
    sys.modules["antenv.axon_hooks"] = mod


_ensure_ntff_hook()

# Problem shapes (hardcoded per contract)
B, S, DIN, DOUT = 2, 2048, 4096, 11008
NCORES = 8
TOK = B * S                      # 4096 tokens
DSH = DOUT // NCORES             # 1376 output features per core
P = 128
KC = DIN // P                    # 32 contraction chunks of 128
MT = TOK // P                    # 32 token tiles of 128
N_TILE = 512
N_SIZES = (512, 512, 352)        # n-tiles covering DSH=1376
HEAD_KC = 2                      # leading k-slices loaded via early queues
WARM_N = 128                     # warmup matmul width
WARM_COUNT = 26                  # warmup matmuls before the first real one
WARM_FILL = 24                   # gap-filler warmups after the leading k<2 block

_cached = {}


def build_module(mt=MT, kc=KC, dsh=DSH, n_sizes=N_SIZES, num_devices=NCORES):
    """Build + compile the Bass module (same NEFF for all cores)."""
    nc = bacc.Bacc(
        "TRN2",
        target_bir_lowering=False,
        debug=False,
        num_devices=num_devices,
    )
    fp16 = mybir.dt.float16
    fp32 = mybir.dt.float32

    # DRAM I/O (per-core shapes; layouts pre-arranged on host)
    x_d = nc.dram_tensor("x", (mt, P, kc, P), fp16, kind="ExternalInput")
    w_d = nc.dram_tensor("w", (P, kc, dsh), fp16, kind="ExternalInput")
    b_d = nc.dram_tensor("b", (P, dsh), fp32, kind="ExternalInput")
    o_d = nc.dram_tensor("out", (mt, P, dsh), fp32, kind="ExternalOutput")

    n_off = []
    off = 0
    for ns in n_sizes:
        n_off.append(off)
        off += ns
    assert off == dsh

    # Weight DMA chunk boundaries (in kc units): uniform 2-kc chunks.
    # (A smaller first chunk and other head perturbations measured slower.)
    step = 2 if kc % 2 == 0 else 1
    w_bounds = list(range(0, kc + 1, step))
    # kc index -> (chunk index, offset within chunk)
    k2chunk = []
    for ci in range(len(w_bounds) - 1):
        for kk in range(w_bounds[ci + 1] - w_bounds[ci]):
            k2chunk.append((ci, kk))

    # How many leading token-tiles to k-interleave so PE work overlaps the
    # weight load (each tile is ~18.3us of PE work vs ~30us of w DMA).
    n_group = 2 if mt >= 2 else mt
    hkc = HEAD_KC

    with tile.TileContext(nc) as tc:
        with (
            tc.tile_pool(name="wpool", bufs=1) as wpool,
            tc.tile_pool(name="xpool", bufs=4) as xpool,
            tc.tile_pool(name="opool", bufs=3) as opool,
            tc.tile_pool(name="psum", bufs=2, space="PSUM") as psum_pool,
        ):
            # ---- head ------------------------------------------------------
            # The x head slices ride the GpSimd DMA queue: it is slow
            # (~30GB/s) but issues at ~3.2us, well before Sync's stream
            # starts (~8us), and it takes 128KB off Sync's early window.
            xheads = []
            for g in range(n_group):
                xh = wpool.tile([P, hkc, P], fp16, tag=f"x{g}h")
                nc.gpsimd.dma_start(out=xh[:], in_=x_d.ap()[g][:, 0:hkc, :])
                xheads.append(xh)

            # PE warmup: dummy matmuls on a small zeroed scratch tile so the
            # HAM clock-gate ramps while the head DMAs are in flight.
            # 128-wide so the fill granularity is fine (~60-200ns each).
            warm_src = wpool.tile([P, WARM_N], fp16, tag="warm_src")
            nc.gpsimd.memset(warm_src[:], 0)
            warm_ps = psum_pool.tile([P, WARM_N], fp32, tag="warm", name="warm")
            for _ in range(WARM_COUNT):
                nc.tensor.matmul(
                    warm_ps[:], warm_src[:, :P], warm_src[:], start=True, stop=True
                )

            # w chunk 0 on Sync (its queue group fans out across 16 HW
            # engines at ~280GB/s).  Gate for the first real matmul =
            # x heads (gpsimd, ~64KB each) + w chunk 0 (704KB).
            w0full = wpool.tile([P, hkc, dsh], fp16, tag="w0")
            nc.sync.dma_start(out=w0full[:], in_=w_d.ap()[:, 0:hkc, :])

            # ---- bulk streams on Sync --------------------------------------
            def alloc_xm(m, head=False):
                if head:
                    xm = xpool.tile([P, kc - hkc, P], fp16, tag="xm", name=f"xm{m}")
                    nc.sync.dma_start(out=xm[:], in_=x_d.ap()[m][:, hkc:kc, :])
                else:
                    xm = xpool.tile([P, kc, P], fp16, tag="xm", name=f"xm{m}")
                    nc.sync.dma_start(out=xm[:], in_=x_d.ap()[m])
                return xm

            def alloc_psums(m):
                psums = []
                for n in range(len(n_sizes)):
                    ps_full = psum_pool.tile(
                        [P, N_TILE], fp32, tag=f"ps{n}", name=f"ps{n}_{m}"
                    )
                    psums.append(ps_full[:, : n_sizes[n]])
                return psums

            def w_slice(wt, kk, n):
                return wt[:, kk, n_off[n] : n_off[n] + n_sizes[n]]

            def mm_lhsT(psums, lhsT, k, wt, kk):
                for n in range(len(n_sizes)):
                    nc.tensor.matmul(
                        psums[n],
                        lhsT,
                        w_slice(wt, kk, n),
                        start=(k == 0),
                        stop=(k == kc - 1),
                    )

            def epilogue(m, psums, split_store=False):
                om = opool.tile([P, dsh], fp32, tag="om", name=f"om{m}")
                for n in range(len(n_sizes)):
                    sl = slice(n_off[n], n_off[n] + n_sizes[n])
                    nc.vector.tensor_add(
                        out=om[:, sl], in0=psums[n], in1=bias_sb[:, sl]
                    )
                    if split_store:
                        nc.sync.dma_start(out=o_d.ap()[m][:, sl], in_=om[:, sl])
                if not split_store:
                    nc.sync.dma_start(out=o_d.ap()[m], in_=om[:])

            # DMA issue order on Sync is FIFO: x rest tiles for the leading
            # group, then remaining weight chunks in consumption order; bias
            # is only needed by the first epilogue (~45us in), so it's last.
            w_tiles = [w0full]

            def load_w_chunk(c):
                lo, hi = w_bounds[c], w_bounds[c + 1]
                wt = wpool.tile([P, hi - lo, dsh], fp16, tag=f"w{c}", name=f"w{c}")
                nc.sync.dma_start(out=wt[:], in_=w_d.ap()[:, lo:hi, :])
                w_tiles.append(wt)

            group_xms = [alloc_xm(g, head=True) for g in range(n_group)]
            for c in range(1, len(w_bounds) - 1):
                load_w_chunk(c)

            bias_sb = wpool.tile([P, dsh], fp32, tag="bias")
            nc.sync.dma_start(out=bias_sb[:], in_=b_d.ap())

            # Leading group, k < hkc: gated only on the head tiles + chunk 0.
            group_psums = [alloc_psums(m) for m in range(n_group)]
            for k in range(hkc):
                for g in range(n_group):
                    for n in range(len(n_sizes)):
                        nc.tensor.matmul(
                            group_psums[g][n],
                            xheads[g][:, k, :],
                            w_slice(w0full, k, n),
                            start=(k == 0),
                            stop=False,
                        )
            # Fill the DMA wait before chunk 1 lands with cheap warmup
            # matmuls so the PE clock-ramp doesn't reset (an idle gap here
            # measured ~2us of half-speed matmuls afterwards).
            for _ in range(WARM_FILL):
                nc.tensor.matmul(
                    warm_ps[:], warm_src[:, :P], warm_src[:], start=True, stop=True
                )

            # Leading group, k >= hkc: interleave over k so matmuls consume
            # weight chunks in arrival order across n_group token tiles.
            for k in range(hkc, kc):
                ci, kk = k2chunk[k]
                wt = w_tiles[ci]
                for g in range(n_group):
                    mm_lhsT(group_psums[g], group_xms[g][:, k - hkc, :], k, wt, kk)
            for g in range(n_group):
                epilogue(g, group_psums[g])

            # Steady state
            for m in range(n_group, mt - 1):
                xm = alloc_xm(m)
                psums = alloc_psums(m)
                for k in range(kc):
                    ci, kk = k2chunk[k]
                    mm_lhsT(psums, xm[:, k, :], k, w_tiles[ci], kk)
                epilogue(m, psums)

            # Last tile: n-major with per-n epilogue+store so the tail after
            # the final matmul is only the 352-wide add + store.
            m = mt - 1
            xm = alloc_xm(m)
            psums = alloc_psums(m)
            om = opool.tile([P, dsh], fp32, tag="om", name=f"om{m}")
            for n in range(len(n_sizes)):
                sl = slice(n_off[n], n_off[n] + n_sizes[n])
                for k in range(kc):
                    ci, kk = k2chunk[k]
                    nc.tensor.matmul(
                        psums[n],
                        xm[:, k, :],
                        w_slice(w_tiles[ci], kk, n),
                        start=(k == 0),
                        stop=(k == kc - 1),
                    )
                nc.vector.tensor_add(out=om[:, sl], in0=psums[n], in1=bias_sb[:, sl])
                nc.scalar.dma_start(out=o_d.ap()[m][:, sl], in_=om[:, sl])

    nc.compile()
    return nc


def _get_module():
    if "nc" not in _cached:
        # num_devices=1: no collectives anywhere in the kernel, and the
        # per-NEFF sync machinery is cheapest in single-device form; the
        # SPMD launcher still runs the same NEFF on all 8 cores.
        _cached["nc"] = build_module(num_devices=1)
    return _cached["nc"]


def _prep_inputs(x, w_int8, scale, bias):
    """Host-side shard + layout prep. Returns in_maps for the 8 cores."""
    # x: fold scale, cast fp16, reorder to [m, kp, kc, t]
    xs = x.reshape(TOK, DIN).astype(np.float32) * np.float32(scale)
    xp = xs.reshape(MT, P, KC, P)        # [m, t, kc, kp]
    xp = np.ascontiguousarray(xp.transpose(0, 3, 2, 1), dtype=np.float16)

    in_maps = []
    for c in range(NCORES):
        wsh = w_int8[c * DSH : (c + 1) * DSH]          # [dsh, DIN] int32
        wp = wsh.reshape(DSH, KC, P).transpose(2, 1, 0)  # [kp, kc, dsh]
        wp = np.ascontiguousarray(wp).astype(np.float16)  # ints <=127: exact
        bsh = np.ascontiguousarray(
            np.broadcast_to(bias[c * DSH : (c + 1) * DSH].astype(np.float32), (P, DSH))
        )
        in_maps.append({"x": xp, "w": wp, "b": bsh})
    return in_maps


def kernel(x, w_int8, scale, bias):
    nc = _get_module()
    in_maps = _prep_inputs(
        np.asarray(x), np.asarray(w_int8), np.asarray(scale), np.asarray(bias)
    )
    res = run_bass_kernel_spmd(nc, in_maps, core_ids=list(range(NCORES)))
    outs = [res.results[c]["out"].reshape(TOK, DSH) for c in range(NCORES)]
    full = np.concatenate(outs, axis=1)  # [TOK, DOUT]
    return np.ascontiguousarray(full.reshape(B, S, DOUT), dtype=np.float32)
